# revision 38
# baseline (speedup 1.0000x reference)
"""Trainium2 Bass kernel for nn_AFM_layer (AFM-style pooling model).

Math (from the reference):
    x1 = concat(dense, gather(emb_tables, sparse))            # [B, 221]
    x2 = (x1 (x) x1) @ W1 + b1                                # [B, 221]
    x3 = (x2 (x) x2) @ W2 + b2                                # [B, 221]
    (softmax over a size-1 axis is all-ones, so the "attention" pooling
     reduces to a plain sum over features)
    y  = sigmoid(sum_k(x3) * out_w + out_b)                   # [B, 1]

Device strategy (data-parallel over batch, 8 cores, 256 samples each):
  * The interaction (x (x) x) @ W is symmetrized: only pairs (i, j<=i..)
    i<=j are computed, with W rows pre-combined on host
    (U[(i,j),k] = W3[i,j,k]+W3[j,i,k] for i<j, W3[i,i,k] on the diagonal).
    This halves FLOPs and weight bytes.
  * Pair products are built in SBUF by DVE tensor_scalar (fp16, 4x mode),
    block-transposed [b, p] -> [p, b] via the DMA xbar, and used as matmul
    stationary operands against streamed fp16 U chunks.  The whole layer
    accumulates into a single PSUM tile [128, 221] per batch tile.
  * Embedding gather runs on-device via indirect DMA (one row per
    partition per instruction; 26 per batch tile).
  * out_w is folded into U2; biases enter via a DMA'd broadcast tile and
    the final sigmoid's per-partition bias.
"""

import sys

if "/opt/trn_rl_repo" not in sys.path:
    sys.path.insert(0, "/opt/trn_rl_repo")

import numpy as np

B, D, S, V, E = 2048, 13, 26, 100000, 8
F = D + S * E  # 221
N_CORES = 8
BC = B // N_CORES  # 256 samples per core
NT = BC // 128  # batch tiles per core
FPAD = 224  # x tile padded feature count

GROUP_TARGET = 6144  # pair columns per chunk (rounded up to x128)
ACT_BUILD_MIN_I = 190  # build ops with i >= this run on ScalarE instead of DVE

# Feature order on device is [emb (208) | dense (13)] so that high-i pair
# builds depend only on late-gathered fields; gathers are issued in
# reverse field order and groups are processed descending, letting the
# build/matmul pipeline start while early fields are still gathering.
# PERM maps new feature index -> original feature index.
PERM = np.array(list(range(D, F)) + list(range(D)), dtype=np.int64)
DENSE0 = F - D  # 208: first dense column in the new order


def pair_layout():
    """Upper-triangle pair layout, widths padded to even, grouped in
    chunks of <= GROUP_TARGET columns, each chunk padded to x128.

    Returns (entries, groups, np_total):
      entries: list of (i, col, width) - build op i writes pair columns
               [col, col+width) with values x[:, i] * x[:, i:i+width]
      groups:  list of (e0, e1, col0, ncols) - entry range, first column,
               padded column count of each chunk
    """
    entries = []
    groups = []
    col = 0
    gcol0 = 0
    ge0 = 0
    gi0 = 0
    for i in range(F):
        w = F - i
        w += w & 1
        if col - gcol0 + w > GROUP_TARGET:
            ncols = -(-(col - gcol0) // 128) * 128
            groups.append((ge0, len(entries), gcol0, ncols))
            gcol0 += ncols
            col = gcol0
            ge0 = len(entries)
            gi0 = i
        entries.append((i, col, w))
        col += w
    ncols = -(-(col - gcol0) // 128) * 128
    groups.append((ge0, len(entries), gcol0, ncols))
    if len(groups) >= 2 and groups[-1][3] <= 1024:
        # merge the small trailing group into the previous one (internal
        # padding between them stays; its U rows are zero)
        e0a, _, col0a, _ = groups[-2]
        _, e1b, col0b, ncolsb = groups[-1]
        groups[-2:] = [(e0a, e1b, col0a, (col0b - col0a) + ncolsb)]
    np_total = groups[-1][2] + groups[-1][3]
    return entries, groups, np_total


ENTRIES, GROUPS, NP = pair_layout()


def pack_u_flat(w_mat: np.ndarray, scale: float) -> np.ndarray:
    """Pack one [F*F, F] interaction weight into the padded symmetric pair
    layout [NP, F], fp16."""
    w3 = w_mat.reshape(F, F, F)[np.ix_(PERM, PERM, PERM)]
    u = np.zeros((NP, F), np.float32)
    for i, col, width in ENTRIES:
        njs = F - i  # real j's: j = i .. F-1
        block = w3[i, i:F, :] + w3[i:F, i, :]  # [njs, F]
        block[0] = w3[i, i, :]  # diagonal counted once
        u[col : col + njs] = block
    return (u * scale).astype(np.float16)


def pack_u(w_mat: np.ndarray, scale: float) -> np.ndarray:
    """Blocked per partition so each group DMA is one contiguous run per
    partition: result [128, (NP//128) * F] fp16 with
    result[p, blk*F:(blk+1)*F] = U[blk*128 + p, :]."""
    u = pack_u_flat(w_mat, scale)
    u_perm = u.reshape(NP // 128, 128, F).transpose(1, 0, 2).reshape(128, -1)
    return np.ascontiguousarray(u_perm)


def host_pair_reference(x: np.ndarray) -> np.ndarray:
    """Numpy emulation of the on-device pair build (fp16), for debugging."""
    xp = np.zeros((x.shape[0], FPAD), np.float16)
    xp[:, :F] = x.astype(np.float16)
    pair = np.zeros((x.shape[0], NP), np.float16)
    for i, col, width in ENTRIES:
        pair[:, col : col + width] = (
            xp[:, i : i + 1].astype(np.float32) * xp[:, i : i + width].astype(np.float32)
        ).astype(np.float16)
    return pair


_COMPILED = None


def _build_kernel():
    import concourse.bass as bass
    import concourse.mybir as mybir
    import concourse.tile as tile
    from concourse import bacc

    dt = mybir.dt
    f32, f16, i32 = dt.float32, dt.float16, dt.int32

    nc = bacc.Bacc("TRN2", target_bir_lowering=False, debug=True)

    npb = NP // 128
    dense = nc.declare_dram_parameter("dense", [BC, D], f32, isOutput=False)
    gidx = nc.declare_dram_parameter("gidx", [BC, S], i32, isOutput=False)
    emb2d = nc.declare_dram_parameter("emb2d", [S * V, E], f32, isOutput=False)
    u1 = nc.declare_dram_parameter("u1", [128, npb * F], f16, isOutput=False)
    u2 = nc.declare_dram_parameter("u2", [128, npb * F], f16, isOutput=False)
    b1r = nc.declare_dram_parameter("b1r", [128, F], f32, isOutput=False)
    pb2 = nc.declare_dram_parameter("pb2", [128, 1], f32, isOutput=False)
    y = nc.declare_dram_parameter("y", [BC, 1], f32, isOutput=True)

    n_groups = len(GROUPS)
    max_ncols = max(g[3] for g in GROUPS)

    with tile.TileContext(nc) as tc:
        with (
            tc.tile_pool(name="persist", bufs=1) as persist,
            tc.tile_pool(name="xpool", bufs=1) as xpool,
            tc.tile_pool(name="pair", bufs=6) as pair_pool,
            tc.tile_pool(name="pairt", bufs=4) as pairt_pool,
            tc.tile_pool(name="upool", bufs=2) as upool,
            tc.tile_pool(name="psum", bufs=2, space="PSUM") as psum_pool,
            tc.tile_pool(name="tail", bufs=2) as tail_pool,
        ):
            b1r_sb = persist.tile([128, F], f32)
            nc.gpsimd.dma_start(b1r_sb[:], b1r[:])
            pb2_sb = persist.tile([128, 1], f32)
            nc.gpsimd.dma_start(pb2_sb[:], pb2[:])
            gidx_sb = persist.tile([128, NT, S], i32)
            nc.gpsimd.dma_start(
                gidx_sb[:], gidx[:].rearrange("(t p) s -> p t s", p=128)
            )

            # Pre-warm the sigmoid ACT table (overlaps with the gathers).
            warm = persist.tile([128, 1], f32)
            nc.scalar.activation(
                warm[:], pb2_sb[:], mybir.ActivationFunctionType.Sigmoid
            )


            # Assemble x per batch tile in [emb | dense] order.  Gathers go
            # field-descending with small casts after each landing, so pair
            # builds (which read x[:, i:]) unblock progressively.
            x_f = []
            x_h = []
            x_h1 = []
            for t in range(NT):
                xf = xpool.tile([128, FPAD], f32, tag=f"x_f{t}", name=f"xf{t}")
                xh = xpool.tile([128, FPAD], f16, tag=f"x_h{t}", name=f"xh{t}")
                xh1 = xpool.tile([128, FPAD], f16, tag=f"x_h1{t}", name=f"xh1{t}")
                nc.vector.memset(xf[:, F:FPAD], 0.0)
                nc.gpsimd.dma_start(
                    xf[:, DENSE0:F], dense[t * 128 : (t + 1) * 128, :]
                )
                nc.vector.tensor_copy(xh[:, DENSE0:FPAD], xf[:, DENSE0:FPAD])
                nc.vector.tensor_copy(
                    xh1[:, DENSE0 - 1 : FPAD - 1], xf[:, DENSE0:FPAD]
                )
                x_f.append(xf)
                x_h.append(xh)
                x_h1.append(xh1)
            for s in range(S - 1, -1, -1):
                for t in range(NT):
                    xf, xh, xh1 = x_f[t], x_h[t], x_h1[t]
                    c0 = E * s
                    nc.gpsimd.indirect_dma_start(
                        out=xf[:, c0 : c0 + E],
                        out_offset=None,
                        in_=emb2d[:],
                        in_offset=bass.IndirectOffsetOnAxis(
                            ap=gidx_sb[:, t, s : s + 1], axis=0
                        ),
                    )
                    nc.vector.tensor_copy(xh[:, c0 : c0 + E], xf[:, c0 : c0 + E])
                    if s > 0:
                        nc.vector.tensor_copy(
                            xh1[:, c0 - 1 : c0 + E - 1], xf[:, c0 : c0 + E]
                        )
                    else:
                        nc.vector.tensor_copy(
                            xh1[:, 0 : E - 1], xf[:, 1:E]
                        )

            for L in range(2):
                u_dram = u1 if L == 0 else u2
                psum_acc = [
                    psum_pool.tile([128, F], f32, tag=f"acc{t}", name=f"acc{L}_{t}")
                    for t in range(NT)
                ]
                proc_order = list(range(n_groups - 1, -1, -1))
                for gi, g in enumerate(proc_order):
                    e0, e1, col0, ncols = GROUPS[g]
                    nblk = ncols // 128
                    blk0 = col0 // 128
                    ug = upool.tile([128, nblk, F], f16, tag="ug")
                    nc.scalar.dma_start(
                        ug[:],
                        u_dram[:, blk0 * F : (blk0 + nblk) * F].rearrange(
                            "p (blk k) -> p blk k", k=F
                        ),
                    )
                    holes = []
                    prev_end = col0
                    for i, col, width in ENTRIES[e0:e1]:
                        if col > prev_end:
                            holes.append((prev_end - col0, col - col0))
                        prev_end = col + width
                    if prev_end < col0 + ncols:
                        holes.append((prev_end - col0, ncols))
                    for t in range(NT):
                        pb = pair_pool.tile([128, max_ncols], f16, tag="pair")
                        for h0, h1 in holes:
                            nc.vector.memset(pb[:, h0:h1], 0.0)
                        for i, col, width in ENTRIES[e0:e1]:
                            c = col - col0
                            if i % 2 == 0:
                                src = x_h[t][:, i : i + width]
                            else:
                                src = x_h1[t][:, i - 1 : i - 1 + width]
                            if i >= ACT_BUILD_MIN_I:
                                nc.scalar.activation(
                                    pb[:, c : c + width],
                                    src,
                                    mybir.ActivationFunctionType.Copy,
                                    scale=x_f[t][:, i : i + 1],
                                )
                            else:
                                nc.vector.tensor_scalar_mul(
                                    pb[:, c : c + width], src, x_f[t][:, i : i + 1]
                                )
                        pT = pairt_pool.tile([128, nblk, 128], f16, tag="pairT")
                        nc.sync.dma_start_transpose(pT[:], pb[:, 0:ncols])
                        for blk in range(nblk):
                            nc.tensor.matmul(
                                psum_acc[t][:],
                                lhsT=pT[:, blk, :],
                                rhs=ug[:, blk, :],
                                start=(gi == 0 and blk == 0),
                                stop=(gi == n_groups - 1 and blk == nblk - 1),
                            )
                # layer epilogue
                for t in range(NT):
                    if L == 0:
                        nc.vector.tensor_add(
                            x_f[t][:, 0:F], psum_acc[t][:], b1r_sb[:]
                        )
                        nc.vector.tensor_copy(x_h[t][:], x_f[t][:])
                        nc.vector.tensor_copy(
                            x_h1[t][:, 0 : FPAD - 1], x_f[t][:, 1:FPAD]
                        )
                    else:
                        pooled = tail_pool.tile([128, 1], f32, tag=f"pool{t}")
                        nc.vector.tensor_reduce(
                            pooled[:],
                            psum_acc[t][:],
                            axis=mybir.AxisListType.X,
                            op=mybir.AluOpType.add,
                        )
                        yt = tail_pool.tile([128, 1], f32, tag=f"yt{t}")
                        nc.scalar.activation(
                            yt[:],
                            pooled[:],
                            mybir.ActivationFunctionType.Sigmoid,
                            bias=pb2_sb[:, 0:1],
                            scale=1.0,
                        )
                        nc.gpsimd.dma_start(y[t * 128 : (t + 1) * 128, :], yt[:])

    nc.compile()
    return nc


def _get_compiled():
    global _COMPILED
    if _COMPILED is None:
        _COMPILED = _build_kernel()
    return _COMPILED


def kernel(
    dense_inputs,
    sparse_inputs,
    emb_tables,
    W1,
    b1,
    W2,
    b2,
    att_w_w,
    att_w_b,
    att_h_w,
    att_h_b,
    out_w,
    out_b,
):
    from concourse.bass_utils import run_bass_kernel_spmd

    nc = _get_compiled()

    dense_inputs = np.asarray(dense_inputs, np.float32)
    sparse_inputs = np.asarray(sparse_inputs, np.int32)
    emb_tables = np.asarray(emb_tables, np.float32)
    ow = float(np.asarray(out_w).reshape(-1)[0])
    ob = float(np.asarray(out_b).reshape(-1)[0])

    emb2d = np.ascontiguousarray(emb_tables.reshape(S * V, E))
    gidx_all = (sparse_inputs + (np.arange(S, dtype=np.int32) * V)[None, :]).astype(
        np.int32
    )
    u1 = pack_u(np.asarray(W1, np.float32), 1.0)
    u2 = pack_u(np.asarray(W2, np.float32), ow)
    b1r = np.ascontiguousarray(
        np.tile(np.asarray(b1, np.float32)[PERM][None, :], (128, 1))
    )
    pb2_val = float(np.sum(np.asarray(b2, np.float32)) * ow + ob)
    pb2 = np.full((128, 1), pb2_val, np.float32)

    in_maps = []
    for c in range(N_CORES):
        sl = slice(c * BC, (c + 1) * BC)
        in_maps.append(
            {
                "dense": np.ascontiguousarray(dense_inputs[sl]),
                "gidx": np.ascontiguousarray(gidx_all[sl]),
                "emb2d": emb2d,
                "u1": u1,
                "u2": u2,
                "b1r": b1r,
                "pb2": pb2,
            }
        )

    res = run_bass_kernel_spmd(nc, in_maps, list(range(N_CORES)))
    y = np.concatenate([res.results[c]["y"] for c in range(N_CORES)], axis=0)
    return y.astype(np.float32)
